# revision 7
# baseline (speedup 1.0000x reference)
"""Trainium2 Bass kernel for the CAM (channel-attention) module — v2.

Reference computation (per batch b):
    energy  = x @ x.T                      # [C, C], contraction over N
    att     = softmax(rowmax(energy) - energy, axis=-1)
            = softmin of energy rows       # (the rowmax cancels in softmax)
    out     = gamma * (att @ x) + x

Shapes: x [B=16, C=64, N=65536] f32, gamma [1] f32.

Sharding: data-parallel over batch across 8 cores (2 batches per core).

Per-core layout trick: each batch's [64, 65536] slab is viewed as
[128, 32768] with partition p = h*64 + c  (h = which half of N).  The energy
splits as E = E_h0 + E_h1 where each half is a [64, 64] Gram matrix over its
half of N, and the apply phase uses a 128x128 block-diagonal attention.

v2 design (vs v1): x is loaded ONCE per batch as bf16 via SWDGE cast-DMA
(fp32 HBM -> bf16 SBUF, cast in the DMA datapath) into a resident
[128, 32768] slab — no separate cast ops and half the SBUF footprint.  The
"+ x" residual is folded into the apply matmul by adding I to the
block-diagonal stationary operand: out = (gamma*att_bd + I) @ x_bf16, so
pass 2 is just matmul + PSUM->SBUF copy + store.  Output is bf16-rounded x
when gamma==0 (rel err ~2e-3, well inside the 2e-2 gate).

Pipeline per batch:
  pass 1: SWDGE cast-load chunk; transpose 128x128 tiles via matmuls against
          identity; copy transposed tiles PSUM->SBUF as bf16 (split ACT/DVE);
          Gram matmuls accumulate E_h0/E_h1 in PSUM, lagged one group so the
          PE never stalls on the copies.
  softmax: E = E_h0 + E_h1 -> softmin rows -> att * gamma -> transpose via
          col-tiled matmuls -> block-diag + I -> bf16 lhsT.
  pass 2: psO = bd_aug^T @ x_bf16 (full output incl. residual), copy
          PSUM->SBUF fp32 (split ACT/DVE), DMA out (alternating HWDGE rings).

HBM traffic is the 32 MB read + 32 MB write per-core minimum; the DMA-only
microbench of this pattern measures ~196-201 us (~335 GB/s), which is the
target.  Batches are software-pipelined: batch i's pass 2 (stores) is
emitted interleaved chunk-by-chunk with batch i+1's pass 1 (loads).
"""

import numpy as np
import ml_dtypes

import concourse.bass as bass
import concourse.bacc as bacc
import concourse.mybir as mybir
import concourse.tile as tile

F32 = mybir.dt.float32
BF16 = mybir.dt.bfloat16

# Full-problem constants (hardcoded per the grading contract).
B_FULL = 16
C = 64
N_FULL = 65536
N_CORES = 8
B_CORE = B_FULL // N_CORES  # 2 batches per core
H = 2                       # N-halves packed into partitions
P = H * C                   # 128 partitions
NV_FULL = N_FULL // H       # 32768 view columns per batch

KT = 128      # transpose/Gram K-tile (partition-dim contraction size)
PSW = 1024    # transpose super-group: 8 K-tiles, 2 PSUM banks, 1 copy
OUT_TILE = 512  # pass-2 matmul free size (one PSUM bank of fp32)


def build_nc(b_core=B_CORE, nv=NV_FULL, chunk=2048, osb_bufs=4, reps=1,
             loop_reps=None, swdge_queues=1):
    """Build the per-core Bass module. x input is host-packed [b, 128, nv].

    reps>1 repeats the whole computation (identical output) — used by the
    timing harness to cancel per-call dispatch overhead via t(R) - t(1).
    """
    assert chunk % PSW == 0 and PSW % KT == 0 and nv % chunk == 0
    assert chunk % OUT_TILE == 0

    nc = bacc.Bacc("TRN2", target_bir_lowering=False,
                   num_swdge_queues=swdge_queues)
    x_d = nc.dram_tensor("x", [b_core, P, nv], F32, kind="ExternalInput")
    ident_d = nc.dram_tensor("ident", [P, P], BF16, kind="ExternalInput")
    gamma_d = nc.dram_tensor("gamma64", [C, 1], F32, kind="ExternalInput")
    out_d = nc.dram_tensor("out", [b_core, P, nv], F32, kind="ExternalOutput")

    nchunks = nv // chunk
    kt_total = nv // KT

    with tile.TileContext(nc) as tc:
        with (
            tc.tile_pool(name="consts", bufs=1) as consts,
            tc.tile_pool(name="xb", bufs=2) as xb_pool,
            tc.tile_pool(name="xtg", bufs=2) as xtg_pool,
            tc.tile_pool(name="osb", bufs=osb_bufs) as osb_pool,
            tc.tile_pool(name="small", bufs=2) as small,
            tc.tile_pool(name="psT", bufs=2, space=bass.MemorySpace.PSUM) as psT_pool,
            tc.tile_pool(name="psE", bufs=1, space=bass.MemorySpace.PSUM) as psE_pool,
            tc.tile_pool(name="psO", bufs=2, space=bass.MemorySpace.PSUM) as psO_pool,
        ):
            ident_sb = consts.tile([P, P], BF16, tag="ident")
            nc.sync.dma_start(ident_sb[:], ident_d[:])
            gam = consts.tile([C, 1], F32, tag="gam")
            nc.sync.dma_start(gam[:], gamma_d[:])

            # Gram (E) matmuls for a transposed super-group are deferred by
            # one group so the PE never stalls on the PSUM->SBUF copy: PE
            # order becomes T(g) T(g+1) E(g) T(g+2) E(g+1) ...
            pending_e = []  # (xtg tile, kt0, psE0, psE1)

            def emit_e_group(xtg, kt0, psE0, psE1):
                for k in range(PSW // KT):
                    st = kt0 + k == 0
                    sp = kt0 + k == kt_total - 1
                    t0 = xtg[:, k * KT:k * KT + C]
                    t1 = xtg[:, k * KT + C:k * KT + 2 * C]
                    nc.tensor.matmul(psE0[:], t0, t0, start=st, stop=sp,
                                     skip_group_check=True)
                    nc.tensor.matmul(psE1[:], t1, t1, start=st, stop=sp,
                                     skip_group_check=True)

            def flush_pending_e():
                while pending_e:
                    emit_e_group(*pending_e.pop(0))

            def emit_pass1_chunk(b, ci, xb, psE0, psE1):
                """Cast-load chunk ci of batch b into the resident bf16 slab,
                transpose, Gram-accumulate."""
                sl = slice(ci * chunk, (ci + 1) * chunk)
                nc.gpsimd.dma_start(xb[:, sl], x_d[b][:, sl])

                for g in range(chunk // PSW):
                    psT = psT_pool.tile([P, PSW], F32, tag="psT")
                    for k in range(PSW // KT):
                        col = ci * chunk + g * PSW + k * KT
                        nc.tensor.matmul(
                            psT[:, k * KT:(k + 1) * KT],
                            xb[:, col:col + KT],
                            ident_sb[:],
                            start=True, stop=True,
                        )
                    xtg = xtg_pool.tile([P, PSW], BF16, tag="xtg")
                    hf = PSW // 2
                    nc.scalar.copy(xtg[:, 0:hf], psT[:, 0:hf])
                    nc.vector.tensor_copy(xtg[:, hf:PSW], psT[:, hf:PSW])
                    kt0 = ci * (chunk // KT) + g * (PSW // KT)
                    pending_e.append((xtg, kt0, psE0, psE1))
                    if len(pending_e) > 1:
                        emit_e_group(*pending_e.pop(0))

            def emit_softmax(psE0, psE1):
                """E = E_h0 + E_h1 -> softmin rows * gamma -> block-diag
                (gamma*att)^T + I as bf16 lhsT for pass 2."""
                e1sb = small.tile([C, C], F32, tag="e1sb")
                nc.scalar.copy(e1sb[:], psE1[:])
                E = small.tile([C, C], F32, tag="E")
                nc.vector.tensor_add(E[:], psE0[:], e1sb[:])

                mn = small.tile([C, 1], F32, tag="mn")
                nc.vector.tensor_reduce(mn[:], E[:], axis=mybir.AxisListType.X,
                                        op=mybir.AluOpType.min)
                pexp = small.tile([C, C], F32, tag="pexp")
                ssum = small.tile([C, 1], F32, tag="ssum")
                nc.scalar.activation(pexp[:], E[:],
                                     mybir.ActivationFunctionType.Exp,
                                     bias=mn[:], scale=-1.0, accum_out=ssum[:])
                rec = small.tile([C, 1], F32, tag="rec")
                nc.vector.reciprocal(rec[:], ssum[:])
                rg = small.tile([C, 1], F32, tag="rg")
                nc.vector.tensor_mul(rg[:], rec[:], gam[:])
                attg = small.tile([C, C], BF16, tag="attg")
                nc.vector.tensor_scalar_mul(attg[:], pexp[:], rg[:])

                # psA shares the psO slots (idle between batches).
                psA = psO_pool.tile([P, P], F32, tag="psO")
                nc.vector.memset(psA[0:C, C:P], 0.0)
                nc.vector.memset(psA[C:P, 0:C], 0.0)
                nc.tensor.matmul(psA[0:C, 0:C], attg[:], ident_sb[0:C, 0:C],
                                 start=True, stop=True)
                nc.tensor.matmul(psA[C:P, C:P], attg[:], ident_sb[0:C, 0:C],
                                 start=True, stop=True)
                # bd_aug = (gamma*att_bd)^T + I  — folds the "+x" residual
                # into the pass-2 matmul.
                bd = small.tile([P, P], BF16, tag="bd")
                nc.vector.tensor_add(bd[:], psA[:], ident_sb[:])
                return bd

            def emit_pass2_chunk(b, ci, xb, bd, qi):
                """psO = bd^T @ x_bf16 (= gamma*att@x + x), copy to SBUF,
                store."""
                ot = osb_pool.tile([P, chunk], F32, tag="osb")
                for s in range(chunk // OUT_TILE):
                    sl = slice(s * OUT_TILE, (s + 1) * OUT_TILE)
                    col = ci * chunk + s * OUT_TILE
                    psO = psO_pool.tile([P, OUT_TILE], F32, tag="psO")
                    nc.tensor.matmul(psO[:], bd[:], xb[:, col:col + OUT_TILE],
                                     start=True, stop=True)
                    if s % 2 == 0:
                        nc.scalar.copy(ot[:, sl], psO[:])
                    else:
                        nc.vector.tensor_copy(ot[:, sl], psO[:])
                # split the store across both HWDGE rings so each chunk's
                # write bandwidth comes from two queues
                hc = chunk // 2
                c0 = ci * chunk
                e0, e1 = (nc.sync, nc.scalar) if qi % 2 == 0 else (
                    nc.scalar, nc.sync)
                e0.dma_start(out_d[b][:, c0:c0 + hc], ot[:, 0:hc])
                e1.dma_start(out_d[b][:, c0 + hc:c0 + chunk], ot[:, hc:chunk])

            # Two-stage software pipeline over the flat batch sequence:
            # batch i's pass-2 (stores, light compute) is emitted interleaved
            # chunk-by-chunk with batch i+1's pass-1 (loads, heavy compute).
            def emit_all(n_batches):
                prev = None  # (b, xb, bd) of batch awaiting pass 2
                for b_rep in range(n_batches):
                    b = b_rep % b_core
                    psE0 = psE_pool.tile([C, C], F32, tag="psE0")
                    psE1 = psE_pool.tile([C, C], F32, tag="psE1")
                    xb = xb_pool.tile([P, nv], BF16, tag="xb")
                    for ci in range(nchunks):
                        if prev is not None:
                            pb, pxb, pbd = prev
                            emit_pass2_chunk(pb, ci, pxb, pbd, ci)
                        emit_pass1_chunk(b, ci, xb, psE0, psE1)
                    flush_pending_e()
                    bd = emit_softmax(psE0, psE1)
                    prev = (b, xb, bd)
                pb, pxb, pbd = prev
                for ci in range(nchunks):
                    emit_pass2_chunk(pb, ci, pxb, pbd, ci)

            if loop_reps is not None:
                # hardware loop of self-contained passes — used by the timing
                # harness (one NEFF executes the kernel loop_reps times)
                with tc.For_i(0, loop_reps, 1):
                    emit_all(b_core)
            else:
                emit_all(b_core * reps)

    nc.compile()
    return nc


def pack_inputs(x_core, gamma):
    """x_core [b, C, N] f32 -> h-major view [b, 128, N//2], plus constants."""
    b = x_core.shape[0]
    n = x_core.shape[2]
    xv = np.ascontiguousarray(
        x_core.reshape(b, C, H, n // H).transpose(0, 2, 1, 3)
    ).reshape(b, P, n // H)
    ident = np.eye(P, dtype=ml_dtypes.bfloat16)
    g64 = np.broadcast_to(np.asarray(gamma, np.float32).reshape(1, 1), (C, 1))
    return {
        "x": xv,
        "ident": ident,
        "gamma64": np.ascontiguousarray(g64),
    }


def unpack_output(out_view, n):
    """[b, 128, n//2] h-major view -> [b, C, n]."""
    b = out_view.shape[0]
    return np.ascontiguousarray(
        out_view.reshape(b, H, C, n // H).transpose(0, 2, 1, 3)
    ).reshape(b, C, n)


_NC_CACHE = {}

# Last BassKernelResults from kernel() — lets a test harness read
# exec_time_ns when run with BASS_TRACE=1.
LAST_RESULTS = None


def kernel(x, gamma):
    from concourse import bass_utils

    x = np.asarray(x, dtype=np.float32)
    gamma = np.asarray(gamma, dtype=np.float32)
    assert x.shape == (B_FULL, C, N_FULL), x.shape

    key = "full"
    if key not in _NC_CACHE:
        _NC_CACHE[key] = build_nc()
    nc = _NC_CACHE[key]

    in_maps = []
    for core in range(N_CORES):
        x_core = x[core * B_CORE:(core + 1) * B_CORE]
        in_maps.append(pack_inputs(x_core, gamma))

    res = bass_utils.run_bass_kernel_spmd(
        nc, in_maps, core_ids=list(range(N_CORES))
    )
    global LAST_RESULTS
    LAST_RESULTS = res
    outs = [unpack_output(r["out"], N_FULL) for r in res.results]
    return np.concatenate(outs, axis=0)
